# revision 22
# baseline (speedup 1.0000x reference)
"""MiniCausalAttention on 8 NeuronCores (Trainium2, Bass/Tile).

Problem: x[4,2048,1024] fp32; q/k/v = x@w+b; causal softmax(q k^T/sqrt(D)) @ v.

Sharding: 8 cores = (batch b in 0..3) x (half h in 0..1). Core (b,h) handles
query tiles t' = 2t+h for t in 0..7 (interleaved 128-row tiles), so every
core sees the SAME set of causal key-extents nk(t) = 256*(t+1) -> one SPMD
program, perfectly balanced.

Projection reassociation (exact algebra, host-precomputed M = Wq Wk^T and
u = Wk bq):
  scores  S = q k^T = (x_q M + 1 (x) u^T) x^T   (per-query terms that cancel
          in softmax are dropped)
  output  O = P_norm v = [(P x) Wv] / rowsum + bv
so neither K nor V is ever materialized. The key-bias u is folded into G':
G'^T = M^T x_q^T + u (x) 1^T via the per-partition bias slot of the
PSUM->SBUF affine, so S = G' x^T needs no extra matmuls at all.

All matmuls run in bf16 (full PE rate); PSUM accumulation fp32; softmax
statistics fp32. G'^T/x^T are [d_model partition, token free] so S comes out
query-major; x rows are token-major for Z = P x after a PE transpose of each
128x128 P tile; Z is PE-transposed again for O = Z Wv.

DMA plumbing: three rings. scalar(Activation)-HWDGE carries the phase-A
critical tensors (bvr, u, M and x_q^T interleaved per 128-block, mask);
sync(SP)-HWDGE carries x^T (key-major) and the output tiles; gpsimd SWDGE
carries x rows and Wv. Phase A splits the K=1024 contraction into two
4-block passes so the PE starts after only half the M/x_q^T stream has
landed; warmup matmuls (which also build 1 (x) bv) cover the first ~6 us.
"""

import sys

if "/opt/trn_rl_repo" not in sys.path:
    sys.path.insert(0, "/opt/trn_rl_repo")

import numpy as np
import ml_dtypes

import concourse.bass as bass  # noqa: F401
import concourse.tile as tile
from concourse import bacc, mybir
from concourse.bass_utils import run_bass_kernel_spmd
from concourse.masks import make_identity

BF16 = mybir.dt.bfloat16
F32 = mybir.dt.float32
AF = mybir.ActivationFunctionType

B, L, D = 4, 2048, 1024
P = 128
NQT = 8          # q-tiles per core, 128 rows each
SCALE = 1.0 / 32.0   # 1/sqrt(D)
NEG = -1.0e30
NWARM = 4        # PE warmup rounds spanning the head DMA wait

_CACHED = {}


def build_nc():
    nc = bacc.Bacc(None, target_bir_lowering=False)

    xt = nc.declare_dram_parameter("xt", [D, L], BF16, isOutput=False)    # x^T
    xr = nc.declare_dram_parameter("xr", [L, D], BF16, isOutput=False)    # x rows
    xtq = nc.declare_dram_parameter("xtq", [D, D], BF16, isOutput=False)  # q cols of x^T
    mm_w = nc.declare_dram_parameter("mm_w", [D, D], BF16, isOutput=False)  # Wq Wk^T
    wv = nc.declare_dram_parameter("wv", [D, D], BF16, isOutput=False)
    um = nc.declare_dram_parameter("um", [P, 8], F32, isOutput=False)     # Wk bq
    bvr = nc.declare_dram_parameter("bvr", [1, D], BF16, isOutput=False)
    mask = nc.declare_dram_parameter("mask", [P, 256], F32, isOutput=False)
    out = nc.declare_dram_parameter("out", [D, D], BF16, isOutput=True)

    with tile.TileContext(nc) as tc:
        with tc.tile_pool(name="persist", bufs=1) as persist:
            xt_sb = persist.tile([P, 8, L], BF16)    # x^T: [d-part, ct, token]
            xr_sb = persist.tile([P, 16, D], BF16)   # x: [tok-part, tt, d]
            gt_sb = persist.tile([P, 8, D], BF16)    # G'^T: [d-part, dt, qcol]
            xtq_sb = persist.tile([P, 8, D], BF16)
            m_sb = persist.tile([P, 8, D], BF16)
            wv_sb = persist.tile([P, 8, D], BF16)
            g1_sb = persist.tile([P, 8, 512], F32)   # phase-A pass-1 partials
            um_sb = persist.tile([P, 8], F32)
            bvr_sb = persist.tile([1, D], BF16)
            mask_sb = persist.tile([P, 256], F32)
            ident = persist.tile([P, P], BF16)
            ones_sb = persist.tile([1, P], BF16)
            ones_row = persist.tile([1, 512], BF16)  # DMA-free warmup operand
            bvb_sb = persist.tile([P, D], F32)       # broadcast bias 1 (x) bv

            make_identity(nc, ident)
            nc.vector.memset(ones_sb, 1.0)
            nc.vector.memset(ones_row, 1.0)

            # DMA issue costs ~600ns of sequencer time PER dma_start, and an
            # engine pops its ready work strictly in emission order — so the
            # WHOLE priority-ordered input stream goes on the sync ring,
            # whose sequencer has nothing else to do until the output tiles.
            # The scalar engine issues NO DMAs (its copies pace phase A).
            nc.sync.dma_start(out=bvr_sb, in_=bvr[:, :])
            nc.sync.dma_start(out=um_sb, in_=um[:, :])
            for i in range(8):
                nc.sync.dma_start(out=m_sb[:, i, :], in_=mm_w[i * P:(i + 1) * P, :])
                nc.sync.dma_start(out=xtq_sb[:, i, :], in_=xtq[i * P:(i + 1) * P, :])
            nc.sync.dma_start(out=mask_sb, in_=mask[:, :])
            for c in range(2):
                for i in range(8):
                    nc.sync.dma_start(out=xt_sb[:, i, c * 1024:(c + 1) * 1024],
                                      in_=xt[i * P:(i + 1) * P, c * 1024:(c + 1) * 1024])
            # gpsimd (SWDGE) carries x rows and Wv, but must not steal queue
            # bandwidth from the critical stream above: each of its DMAs is
            # WAW-gated behind a 2-element vector write that reads the last
            # critical tile (xtq block 7), so descriptor generation only
            # starts once the sync stream has drained.
            for tt in range(16):
                nc.vector.tensor_copy(xr_sb[0:1, tt, 0:2], xtq_sb[0:1, 7, 0:2])
            for i in range(8):
                nc.vector.tensor_copy(wv_sb[0:1, i, 0:2], xtq_sb[0:1, 7, 0:2])
            for tt in range(4):
                nc.gpsimd.dma_start(out=xr_sb[:, tt, :], in_=xr[tt * P:(tt + 1) * P, :])
            for i in range(8):
                nc.gpsimd.dma_start(out=wv_sb[:, i, :], in_=wv[i * P:(i + 1) * P, :])
            for tt in range(4, 16):
                nc.gpsimd.dma_start(out=xr_sb[:, tt, :], in_=xr[tt * P:(tt + 1) * P, :])

            # Two PSUM pools spanning phases A and B (8 banks total):
            # psS: tags s (x2) + ptp (x2); psZ: pz0, pz1, po0, po1.
            with tc.tile_pool(name="bwork", bufs=2) as bwork, \
                 tc.tile_pool(name="psA", bufs=2, space="PSUM") as psS, \
                 tc.tile_pool(name="psC", bufs=1, space="PSUM") as psZ:
                psT = psS   # transposes share the psA pool (tag ptp)
                psO = psZ

                # Warmup matmuls keep the PE busy (and its clock ramping)
                # while the phase-A stream lands; early rounds use a memset
                # constant so no DMA gates them, the last round builds
                # bvb = 1 (x) bv from the real bv row.
                for r in range(NWARM):
                    for dc in range(2):
                        pb = psZ.tile([P, 512], F32, tag=f"po{dc}", name=f"pb{dc}")
                        rhs = (bvr_sb[:, dc * 512:(dc + 1) * 512]
                               if r == NWARM - 1 else ones_row)
                        nc.tensor.matmul(pb, ones_sb, rhs, start=True, stop=True)
                        if r == NWARM - 1:
                            nc.scalar.copy(bvb_sb[:, dc * 512:(dc + 1) * 512], pb)

                # ------- Phase A: G'^T = M^T x_q^T + u (x) 1^T -------------
                # Contraction split 4+4 so pass 1 starts after half the
                # scalar-ring stream; pass 2 adds u via the affine bias slot.
                def filler(n):
                    # DMA-free PE work the static scheduler can slot into
                    # predicted idle while the phase-A stream trickles in.
                    for f in range(n):
                        pb = psZ.tile([P, 512], F32, tag=f"po{f % 2}", name="pf")
                        nc.tensor.matmul(pb, ones_sb, ones_row,
                                         start=True, stop=True)

                for qc in range(2):
                    qsl = slice(qc * 512, (qc + 1) * 512)
                    for dt in range(8):
                        pg = psS.tile([P, 512], F32, tag="s", name="pg")
                        for ct in range(4):
                            nc.tensor.matmul(
                                pg,
                                m_sb[:, ct, dt * P:(dt + 1) * P],
                                xtq_sb[:, ct, qsl],
                                start=(ct == 0),
                                stop=(ct == 3),
                            )
                        nc.vector.tensor_copy(g1_sb[:, dt, :], pg)
                        if qc == 0:
                            filler(1)
                    for dt in range(8):
                        pg = psS.tile([P, 512], F32, tag="s", name="pg2")
                        for ct in range(4, 8):
                            nc.tensor.matmul(
                                pg,
                                m_sb[:, ct, dt * P:(dt + 1) * P],
                                xtq_sb[:, ct, qsl],
                                start=(ct == 4),
                                stop=(ct == 7),
                            )
                        nc.vector.affine_then_add(
                            out=gt_sb[:, dt, qsl],
                            in0=pg,
                            in1=g1_sb[:, dt, :],
                            scale=1.0,
                            bias=um_sb[:, dt:dt + 1],
                        )

                # ------------- Phase B: attention per q-tile ---------------
                # S chunks -> exp (+rowsum) -> P^T -> Z = P x -> Z^T ->
                # O = Z^T.T Wv, scaled by 1/rowsum, + bvb.
                for t in range(NQT):
                    nk = 256 * (t + 1)
                    nkc = (nk + 511) // 512  # 512-chunks (last may be 256)

                    p_sb = bwork.tile([P, 2048], BF16, tag="p")
                    rsum = bwork.tile([P, 4], F32, tag="rsum")
                    pz = [psZ.tile([P, 512], F32, tag=f"pz{dc}", name=f"pz{dc}")
                          for dc in range(2)]

                    # For the last tile, process the diagonal chunk FIRST so
                    # its mask+exp chain hides under the other chunks' work
                    # instead of stalling the PE at the kernel tail.
                    corder = (list(range(nkc)) if t < NQT - 1
                              else [nkc - 1] + list(range(nkc - 1)))
                    first_kt = corder[0] * 4
                    lastc = corder[-1]
                    last_kt = lastc * 4 + min(512, nk - lastc * 512) // P - 1
                    for c in corder:
                        w = min(512, nk - c * 512)
                        ps = psS.tile([P, 512], F32, tag="s")
                        for dt in range(8):
                            nc.tensor.matmul(
                                ps[:, :w],
                                gt_sb[:, dt, t * P:(t + 1) * P],
                                xt_sb[:, dt, c * 512:c * 512 + w],
                                start=(dt == 0),
                                stop=(dt == 7),
                            )
                        if c == nkc - 1:
                            nc.vector.tensor_add(ps[:, w - 256:w],
                                                 ps[:, w - 256:w], mask_sb)
                        nc.scalar.activation(
                            p_sb[:, c * 512:c * 512 + w], ps[:, :w], AF.Exp,
                            scale=SCALE, accum_out=rsum[:, c:c + 1])
                        # All transposes+copies of this chunk first (copies
                        # alternate scalar/vector) so every Z matmul's
                        # LDWEIGHTS is prefetched instead of exposed.
                        kts = list(range(c * 4, c * 4 + w // P))
                        pts = {}
                        for j, kt in enumerate(kts):
                            ptp = psT.tile([P, P], BF16, tag="ptp")
                            nc.tensor.transpose(
                                ptp, p_sb[:, kt * P:(kt + 1) * P], ident)
                            pt_sb = bwork.tile([P, P], BF16, tag="pt", bufs=4)
                            if j % 2 == 0:
                                nc.vector.tensor_copy(pt_sb, ptp)
                            else:
                                nc.scalar.copy(pt_sb, ptp)
                            pts[kt] = pt_sb
                        for kt in kts:
                            for dc in range(2):
                                nc.tensor.matmul(
                                    pz[dc],
                                    pts[kt],
                                    xr_sb[:, kt, dc * 512:(dc + 1) * 512],
                                    start=(kt == first_kt),
                                    stop=(kt == last_kt),
                                )

                    # Z -> SBUF (bf16) in 256-col chunks split across the
                    # scalar+vector engines (keeps exp off the copy queue),
                    # then Z^T tiles, then O = Z Wv
                    z_sb = bwork.tile([P, D], BF16, tag="z")
                    for zc in range(4):
                        src = pz[zc // 2][:, (zc % 2) * 256:(zc % 2) * 256 + 256]
                        dst = z_sb[:, zc * 256:(zc + 1) * 256]
                        if zc % 2 == 0:
                            nc.vector.tensor_copy(dst, src)
                        else:
                            nc.scalar.copy(dst, src)
                    po = [psO.tile([P, 512], F32, tag=f"po{dc}", name=f"po{dc}")
                          for dc in range(2)]
                    zts = []
                    for cc in range(8):
                        ztp = psT.tile([P, P], BF16, tag="ptp", name="ztp")
                        nc.tensor.transpose(ztp, z_sb[:, cc * P:(cc + 1) * P], ident)
                        zt_sb = bwork.tile([P, P], BF16, tag="zt", bufs=8)
                        if cc % 2 == 0:
                            nc.vector.tensor_copy(zt_sb, ztp)
                        else:
                            nc.scalar.copy(zt_sb, ztp)
                        zts.append(zt_sb)

                    rinv = bwork.tile([P, 1], F32, tag="rinv")
                    rtot = bwork.tile([P, 1], F32, tag="rtot")
                    nc.vector.reduce_sum(rtot, rsum[:, :nkc], axis=mybir.AxisListType.X)
                    nc.vector.reciprocal(rinv, rtot)
                    o_sb = bwork.tile([P, D], BF16, tag="o")
                    if t < NQT - 1:
                        for cc in range(8):
                            for dc in range(2):
                                nc.tensor.matmul(
                                    po[dc], zts[cc],
                                    wv_sb[:, cc, dc * 512:(dc + 1) * 512],
                                    start=(cc == 0), stop=(cc == 7))
                        for dc in range(2):
                            sl = slice(dc * 512, (dc + 1) * 512)
                            nc.vector.affine_then_add(
                                out=o_sb[:, sl], in0=po[dc], in1=bvb_sb[:, sl],
                                scale=rinv, bias=0.0)
                            nc.sync.dma_start(out=out[t * P:(t + 1) * P, sl],
                                              in_=o_sb[:, sl])
                    else:
                        # Last tile: finish po[0] first so its normalize +
                        # store overlap po[1]'s matmuls (shorter tail).
                        for dc in range(2):
                            for cc in range(8):
                                nc.tensor.matmul(
                                    po[dc], zts[cc],
                                    wv_sb[:, cc, dc * 512:(dc + 1) * 512],
                                    start=(cc == 0), stop=(cc == 7))
                            sl = slice(dc * 512, (dc + 1) * 512)
                            nc.vector.affine_then_add(
                                out=o_sb[:, sl], in0=po[dc], in1=bvb_sb[:, sl],
                                scale=rinv, bias=0.0)
                            nc.sync.dma_start(out=out[t * P:(t + 1) * P, sl],
                                              in_=o_sb[:, sl])

    nc.finalize()
    return nc


def _prep_inputs(x, wq, bq, wk, bk, wv, bv):
    bf = ml_dtypes.bfloat16
    wq32 = np.asarray(wq, np.float32)
    wk32 = np.asarray(wk, np.float32)
    m_host = (wq32 @ wk32.T).astype(bf)                 # Wq Wk^T
    u_host = (wk32 @ np.asarray(bq, np.float32))        # Wk bq, [D]
    um = np.ascontiguousarray(u_host.reshape(8, P).T).astype(np.float32)
    wv_b = np.ascontiguousarray(wv, np.float32).astype(bf)
    bvr = np.asarray(bv, np.float32).reshape(1, D).astype(bf)

    i = np.arange(P)[:, None]
    j = np.arange(256)[None, :]
    masks = [np.where(j <= i + P * h, 0.0, NEG).astype(np.float32)
             for h in range(2)]

    in_maps = []
    for core in range(8):
        b, h = core // 2, core % 2
        xb = np.asarray(x[b], np.float32)
        xT = np.ascontiguousarray(xb.T).astype(bf)
        xR = xb.astype(bf)
        qcols = (np.arange(8)[:, None] * 2 + h) * P + np.arange(P)[None, :]
        xTq = np.ascontiguousarray(xT[:, qcols.ravel()])
        in_maps.append({
            "xt": xT, "xr": xR, "xtq": xTq, "mm_w": m_host, "wv": wv_b,
            "um": um, "bvr": bvr, "mask": masks[h],
        })
    return in_maps


def kernel(x, wq, bq, wk, bk, wv, bv, _trace=False, _trace_kwargs=None):
    if "nc" not in _CACHED:
        _CACHED["nc"] = build_nc()
    nc = _CACHED["nc"]
    in_maps = _prep_inputs(x, wq, bq, wk, bk, wv, bv)
    kw = {}
    if _trace:
        kw = dict(trace=True, **(_trace_kwargs or {}))
    res = run_bass_kernel_spmd(nc, in_maps, list(range(8)), **kw)
    out = np.empty((B, L, D), np.float32)
    for core in range(8):
        b, h = core // 2, core % 2
        o = np.asarray(res.results[core]["out"], np.float32)
        out[b].reshape(16, P, D)[h::2] = o.reshape(NQT, P, D)
    if _trace:
        _CACHED["last_results"] = res
    return out


# revision 24
# speedup vs baseline: 1.0094x; 1.0094x over previous
"""MiniCausalAttention on 8 NeuronCores (Trainium2, Bass/Tile).

Problem: x[4,2048,1024] fp32; q/k/v = x@w+b; causal softmax(q k^T/sqrt(D)) @ v.

Sharding: 8 cores = (batch b in 0..3) x (half h in 0..1). Core (b,h) handles
query tiles t' = 2t+h for t in 0..7 (interleaved 128-row tiles), so every
core sees the SAME set of causal key-extents nk(t) = 256*(t+1) -> one SPMD
program, perfectly balanced.

Projection reassociation (exact algebra, host-precomputed M = Wq Wk^T and
u = Wk bq):
  scores  S = q k^T = (x_q M + 1 (x) u^T) x^T   (per-query terms that cancel
          in softmax are dropped)
  output  O = P_norm v = [(P x) Wv] / rowsum + bv
so neither K nor V is ever materialized. The key-bias u is folded into G':
G'^T = M^T x_q^T + u (x) 1^T via the per-partition bias slot of the
PSUM->SBUF affine, so S = G' x^T needs no extra matmuls at all.

All matmuls run in bf16 (full PE rate); PSUM accumulation fp32; softmax
statistics fp32. G'^T/x^T are [d_model partition, token free] so S comes out
query-major; x rows are token-major for Z = P x after a PE transpose of each
128x128 P tile; Z is PE-transposed again for O = Z Wv.

DMA plumbing: three rings. scalar(Activation)-HWDGE carries the phase-A
critical tensors (bvr, u, M and x_q^T interleaved per 128-block, mask);
sync(SP)-HWDGE carries x^T (key-major) and the output tiles; gpsimd SWDGE
carries x rows and Wv. Phase A splits the K=1024 contraction into two
4-block passes so the PE starts after only half the M/x_q^T stream has
landed; warmup matmuls (which also build 1 (x) bv) cover the first ~6 us.
"""

import sys

if "/opt/trn_rl_repo" not in sys.path:
    sys.path.insert(0, "/opt/trn_rl_repo")

import numpy as np
import ml_dtypes

import concourse.bass as bass  # noqa: F401
import concourse.tile as tile
from concourse import bacc, mybir
from concourse.bass_utils import run_bass_kernel_spmd
from concourse.masks import make_identity

BF16 = mybir.dt.bfloat16
F32 = mybir.dt.float32
AF = mybir.ActivationFunctionType

B, L, D = 4, 2048, 1024
P = 128
NQT = 8          # q-tiles per core, 128 rows each
SCALE = 1.0 / 32.0   # 1/sqrt(D)
NEG = -1.0e30
NWARM = 5        # PE warmup rounds spanning the head DMA wait

_CACHED = {}


def build_nc():
    nc = bacc.Bacc(None, target_bir_lowering=False)

    xt = nc.declare_dram_parameter("xt", [D, L], BF16, isOutput=False)    # x^T
    xr = nc.declare_dram_parameter("xr", [L, D], BF16, isOutput=False)    # x rows
    xtq = nc.declare_dram_parameter("xtq", [D, D], BF16, isOutput=False)  # q cols of x^T
    mm_w = nc.declare_dram_parameter("mm_w", [D, D], BF16, isOutput=False)  # Wq Wk^T
    wv = nc.declare_dram_parameter("wv", [D, D], BF16, isOutput=False)
    um = nc.declare_dram_parameter("um", [P, 8], F32, isOutput=False)     # Wk bq
    bvr = nc.declare_dram_parameter("bvr", [1, D], BF16, isOutput=False)
    mask = nc.declare_dram_parameter("mask", [P, 256], F32, isOutput=False)
    out = nc.declare_dram_parameter("out", [D, D], BF16, isOutput=True)

    with tile.TileContext(nc) as tc:
        with tc.tile_pool(name="persist", bufs=1) as persist:
            xt_sb = persist.tile([P, 8, L], BF16)    # x^T: [d-part, ct, token]
            xr_sb = persist.tile([P, 16, D], BF16)   # x: [tok-part, tt, d]
            gt_sb = persist.tile([P, 8, D], BF16)    # G'^T: [d-part, dt, qcol]
            xtq_sb = persist.tile([P, 8, D], BF16)
            m_sb = persist.tile([P, 8, D], BF16)
            wv_sb = persist.tile([P, 8, D], BF16)
            g1_sb = persist.tile([P, 8, 512], F32)   # phase-A pass-1 partials
            um_sb = persist.tile([P, 8], F32)
            bvr_sb = persist.tile([1, D], BF16)
            mask_sb = persist.tile([P, 256], F32)
            ident = persist.tile([P, P], BF16)
            ones_sb = persist.tile([1, P], BF16)
            ones_row = persist.tile([1, 512], BF16)  # DMA-free warmup operand
            bvb_sb = persist.tile([P, D], F32)       # broadcast bias 1 (x) bv

            make_identity(nc, ident)
            nc.vector.memset(ones_sb, 1.0)
            nc.vector.memset(ones_row, 1.0)

            # DMA issue costs ~600ns of sequencer time PER dma_start, and an
            # engine pops its ready work strictly in emission order — so the
            # WHOLE priority-ordered input stream goes on the sync ring,
            # whose sequencer has nothing else to do until the output tiles.
            # The scalar engine issues NO DMAs (its copies pace phase A).
            nc.sync.dma_start(out=bvr_sb, in_=bvr[:, :])
            nc.sync.dma_start(out=um_sb, in_=um[:, :])
            for i in range(8):
                nc.sync.dma_start(out=m_sb[:, i, :], in_=mm_w[i * P:(i + 1) * P, :])
                nc.sync.dma_start(out=xtq_sb[:, i, :], in_=xtq[i * P:(i + 1) * P, :])
            nc.sync.dma_start(out=mask_sb, in_=mask[:, :])
            for c in range(2):
                for i in range(8):
                    nc.sync.dma_start(out=xt_sb[:, i, c * 1024:(c + 1) * 1024],
                                      in_=xt[i * P:(i + 1) * P, c * 1024:(c + 1) * 1024])
            # gpsimd (SWDGE) carries x rows and Wv, but must not steal queue
            # bandwidth from the critical stream above: each of its DMAs is
            # WAW-gated behind a 2-element vector write that reads the last
            # critical tile (xtq block 7), so descriptor generation only
            # starts once the sync stream has drained.
            for tt in range(16):
                nc.vector.tensor_copy(xr_sb[0:1, tt, 0:2], xtq_sb[0:1, 7, 0:2])
            for i in range(8):
                nc.vector.tensor_copy(wv_sb[0:1, i, 0:2], xtq_sb[0:1, 7, 0:2])
            for tt in range(4):
                nc.gpsimd.dma_start(out=xr_sb[:, tt, :], in_=xr[tt * P:(tt + 1) * P, :])
            for i in range(8):
                nc.gpsimd.dma_start(out=wv_sb[:, i, :], in_=wv[i * P:(i + 1) * P, :])
            for tt in range(4, 16):
                nc.gpsimd.dma_start(out=xr_sb[:, tt, :], in_=xr[tt * P:(tt + 1) * P, :])

            # Two PSUM pools spanning phases A and B (8 banks total):
            # psS: tags s (x2) + ptp (x2); psZ: pz0, pz1, po0, po1.
            with tc.tile_pool(name="bwork", bufs=2) as bwork, \
                 tc.tile_pool(name="psA", bufs=2, space="PSUM") as psS, \
                 tc.tile_pool(name="psC", bufs=1, space="PSUM") as psZ:
                psT = psS   # transposes share the psA pool (tag ptp)
                psO = psZ

                # Warmup matmuls keep the PE busy (and its clock ramping)
                # while the phase-A stream lands; early rounds use a memset
                # constant so no DMA gates them, the last round builds
                # bvb = 1 (x) bv from the real bv row.
                for r in range(NWARM):
                    for dc in range(2):
                        pb = psZ.tile([P, 512], F32, tag=f"po{dc}", name=f"pb{dc}")
                        rhs = (bvr_sb[:, dc * 512:(dc + 1) * 512]
                               if r == NWARM - 1 else ones_row)
                        nc.tensor.matmul(pb, ones_sb, rhs, start=True, stop=True)
                        if r == NWARM - 1:
                            nc.scalar.copy(bvb_sb[:, dc * 512:(dc + 1) * 512], pb)

                # ------- Phase A: G'^T = M^T x_q^T + u (x) 1^T -------------
                # Contraction split 4+4 so pass 1 starts after half the
                # scalar-ring stream; pass 2 adds u via the affine bias slot.
                for qc in range(2):
                    qsl = slice(qc * 512, (qc + 1) * 512)
                    for dt in range(8):
                        pg = psS.tile([P, 512], F32, tag="s", name="pg")
                        for ct in range(4):
                            nc.tensor.matmul(
                                pg,
                                m_sb[:, ct, dt * P:(dt + 1) * P],
                                xtq_sb[:, ct, qsl],
                                start=(ct == 0),
                                stop=(ct == 3),
                            )
                        nc.vector.tensor_copy(g1_sb[:, dt, :], pg)
                    for dt in range(8):
                        pg = psS.tile([P, 512], F32, tag="s", name="pg2")
                        for ct in range(4, 8):
                            nc.tensor.matmul(
                                pg,
                                m_sb[:, ct, dt * P:(dt + 1) * P],
                                xtq_sb[:, ct, qsl],
                                start=(ct == 4),
                                stop=(ct == 7),
                            )
                        nc.vector.affine_then_add(
                            out=gt_sb[:, dt, qsl],
                            in0=pg,
                            in1=g1_sb[:, dt, :],
                            scale=1.0,
                            bias=um_sb[:, dt:dt + 1],
                        )

                # ------------- Phase B: attention per q-tile ---------------
                # S chunks -> exp (+rowsum) -> P^T -> Z = P x -> Z^T ->
                # O = Z^T.T Wv, scaled by 1/rowsum, + bvb.
                for t in range(NQT):
                    nk = 256 * (t + 1)
                    nkc = (nk + 511) // 512  # 512-chunks (last may be 256)

                    p_sb = bwork.tile([P, 2048], BF16, tag="p")
                    rsum = bwork.tile([P, 4], F32, tag="rsum")
                    pz = [psZ.tile([P, 512], F32, tag=f"pz{dc}", name=f"pz{dc}")
                          for dc in range(2)]

                    # For the last tile, process the diagonal chunk FIRST so
                    # its mask+exp chain hides under the other chunks' work
                    # instead of stalling the PE at the kernel tail.
                    corder = (list(range(nkc)) if t < NQT - 1
                              else [nkc - 1] + list(range(nkc - 1)))
                    first_kt = corder[0] * 4
                    lastc = corder[-1]
                    last_kt = lastc * 4 + min(512, nk - lastc * 512) // P - 1
                    for c in corder:
                        w = min(512, nk - c * 512)
                        ps = psS.tile([P, 512], F32, tag="s")
                        for dt in range(8):
                            nc.tensor.matmul(
                                ps[:, :w],
                                gt_sb[:, dt, t * P:(t + 1) * P],
                                xt_sb[:, dt, c * 512:c * 512 + w],
                                start=(dt == 0),
                                stop=(dt == 7),
                            )
                        if c == nkc - 1:
                            nc.vector.tensor_add(ps[:, w - 256:w],
                                                 ps[:, w - 256:w], mask_sb)
                        nc.scalar.activation(
                            p_sb[:, c * 512:c * 512 + w], ps[:, :w], AF.Exp,
                            scale=SCALE, accum_out=rsum[:, c:c + 1])
                        # All transposes+copies of this chunk first (copies
                        # alternate scalar/vector) so every Z matmul's
                        # LDWEIGHTS is prefetched instead of exposed.
                        kts = list(range(c * 4, c * 4 + w // P))
                        pts = {}
                        for j, kt in enumerate(kts):
                            ptp = psT.tile([P, P], BF16, tag="ptp")
                            nc.tensor.transpose(
                                ptp, p_sb[:, kt * P:(kt + 1) * P], ident)
                            pt_sb = bwork.tile([P, P], BF16, tag="pt", bufs=4)
                            if j % 2 == 0:
                                nc.vector.tensor_copy(pt_sb, ptp)
                            else:
                                nc.scalar.copy(pt_sb, ptp)
                            pts[kt] = pt_sb
                        for kt in kts:
                            for dc in range(2):
                                nc.tensor.matmul(
                                    pz[dc],
                                    pts[kt],
                                    xr_sb[:, kt, dc * 512:(dc + 1) * 512],
                                    start=(kt == first_kt),
                                    stop=(kt == last_kt),
                                )

                    # Z -> SBUF (bf16) in 256-col chunks split across the
                    # scalar+vector engines (keeps exp off the copy queue),
                    # then Z^T tiles, then O = Z Wv
                    z_sb = bwork.tile([P, D], BF16, tag="z")
                    for zc in range(4):
                        src = pz[zc // 2][:, (zc % 2) * 256:(zc % 2) * 256 + 256]
                        dst = z_sb[:, zc * 256:(zc + 1) * 256]
                        if zc % 2 == 0:
                            nc.vector.tensor_copy(dst, src)
                        else:
                            nc.scalar.copy(dst, src)
                    po = [psO.tile([P, 512], F32, tag=f"po{dc}", name=f"po{dc}")
                          for dc in range(2)]
                    zts = []
                    for cc in range(8):
                        ztp = psT.tile([P, P], BF16, tag="ptp", name="ztp")
                        nc.tensor.transpose(ztp, z_sb[:, cc * P:(cc + 1) * P], ident)
                        zt_sb = bwork.tile([P, P], BF16, tag="zt", bufs=8)
                        if cc % 2 == 0:
                            nc.vector.tensor_copy(zt_sb, ztp)
                        else:
                            nc.scalar.copy(zt_sb, ztp)
                        zts.append(zt_sb)

                    rinv = bwork.tile([P, 1], F32, tag="rinv")
                    rtot = bwork.tile([P, 1], F32, tag="rtot")
                    nc.vector.reduce_sum(rtot, rsum[:, :nkc], axis=mybir.AxisListType.X)
                    nc.vector.reciprocal(rinv, rtot)
                    o_sb = bwork.tile([P, D], BF16, tag="o")
                    if t < NQT - 1:
                        for cc in range(8):
                            for dc in range(2):
                                nc.tensor.matmul(
                                    po[dc], zts[cc],
                                    wv_sb[:, cc, dc * 512:(dc + 1) * 512],
                                    start=(cc == 0), stop=(cc == 7))
                        for dc in range(2):
                            sl = slice(dc * 512, (dc + 1) * 512)
                            nc.vector.affine_then_add(
                                out=o_sb[:, sl], in0=po[dc], in1=bvb_sb[:, sl],
                                scale=rinv, bias=0.0)
                            nc.sync.dma_start(out=out[t * P:(t + 1) * P, sl],
                                              in_=o_sb[:, sl])
                    else:
                        # Last tile: finish po[0] first so its normalize +
                        # store overlap po[1]'s matmuls (shorter tail).
                        for dc in range(2):
                            for cc in range(8):
                                nc.tensor.matmul(
                                    po[dc], zts[cc],
                                    wv_sb[:, cc, dc * 512:(dc + 1) * 512],
                                    start=(cc == 0), stop=(cc == 7))
                            sl = slice(dc * 512, (dc + 1) * 512)
                            nc.vector.affine_then_add(
                                out=o_sb[:, sl], in0=po[dc], in1=bvb_sb[:, sl],
                                scale=rinv, bias=0.0)
                            nc.sync.dma_start(out=out[t * P:(t + 1) * P, sl],
                                              in_=o_sb[:, sl])

    nc.finalize()
    return nc


def _prep_inputs(x, wq, bq, wk, bk, wv, bv):
    bf = ml_dtypes.bfloat16
    wq32 = np.asarray(wq, np.float32)
    wk32 = np.asarray(wk, np.float32)
    m_host = (wq32 @ wk32.T).astype(bf)                 # Wq Wk^T
    u_host = (wk32 @ np.asarray(bq, np.float32))        # Wk bq, [D]
    um = np.ascontiguousarray(u_host.reshape(8, P).T).astype(np.float32)
    wv_b = np.ascontiguousarray(wv, np.float32).astype(bf)
    bvr = np.asarray(bv, np.float32).reshape(1, D).astype(bf)

    i = np.arange(P)[:, None]
    j = np.arange(256)[None, :]
    masks = [np.where(j <= i + P * h, 0.0, NEG).astype(np.float32)
             for h in range(2)]

    in_maps = []
    for core in range(8):
        b, h = core // 2, core % 2
        xb = np.asarray(x[b], np.float32)
        xT = np.ascontiguousarray(xb.T).astype(bf)
        xR = xb.astype(bf)
        qcols = (np.arange(8)[:, None] * 2 + h) * P + np.arange(P)[None, :]
        xTq = np.ascontiguousarray(xT[:, qcols.ravel()])
        in_maps.append({
            "xt": xT, "xr": xR, "xtq": xTq, "mm_w": m_host, "wv": wv_b,
            "um": um, "bvr": bvr, "mask": masks[h],
        })
    return in_maps


def kernel(x, wq, bq, wk, bk, wv, bv, _trace=False, _trace_kwargs=None):
    if "nc" not in _CACHED:
        _CACHED["nc"] = build_nc()
    nc = _CACHED["nc"]
    in_maps = _prep_inputs(x, wq, bq, wk, bk, wv, bv)
    kw = {}
    if _trace:
        kw = dict(trace=True, **(_trace_kwargs or {}))
    res = run_bass_kernel_spmd(nc, in_maps, list(range(8)), **kw)
    out = np.empty((B, L, D), np.float32)
    for core in range(8):
        b, h = core // 2, core % 2
        o = np.asarray(res.results[core]["out"], np.float32)
        out[b].reshape(16, P, D)[h::2] = o.reshape(NQT, P, D)
    if _trace:
        _CACHED["last_results"] = res
    return out


# revision 27
# speedup vs baseline: 1.0109x; 1.0015x over previous
"""MiniCausalAttention on 8 NeuronCores (Trainium2, Bass/Tile).

Problem: x[4,2048,1024] fp32; q/k/v = x@w+b; causal softmax(q k^T/sqrt(D)) @ v.

Sharding: 8 cores = (batch b in 0..3) x (half h in 0..1). Core (b,h) handles
query tiles t' = 2t+h for t in 0..7 (interleaved 128-row tiles), so every
core sees the SAME set of causal key-extents nk(t) = 256*(t+1) -> one SPMD
program, perfectly balanced.

Projection reassociation (exact algebra, host-precomputed M = Wq Wk^T and
u = Wk bq):
  scores  S = q k^T = (x_q M + 1 (x) u^T) x^T   (per-query terms that cancel
          in softmax are dropped)
  output  O = P_norm v = [(P x) Wv] / rowsum + bv
so neither K nor V is ever materialized. The key-bias u is folded into G':
G'^T = M^T x_q^T + u (x) 1^T via the per-partition bias slot of the
PSUM->SBUF affine, so S = G' x^T needs no extra matmuls at all.

All matmuls run in bf16 (full PE rate); PSUM accumulation fp32; softmax
statistics fp32. G'^T/x^T are [d_model partition, token free] so S comes out
query-major; x rows are token-major for Z = P x after a PE transpose of each
128x128 P tile; Z is PE-transposed again for O = Z Wv.

DMA plumbing: three rings. scalar(Activation)-HWDGE carries the phase-A
critical tensors (bvr, u, M and x_q^T interleaved per 128-block, mask);
sync(SP)-HWDGE carries x^T (key-major) and the output tiles; gpsimd SWDGE
carries x rows and Wv. Phase A splits the K=1024 contraction into two
4-block passes so the PE starts after only half the M/x_q^T stream has
landed; warmup matmuls (which also build 1 (x) bv) cover the first ~6 us.
"""

import sys

if "/opt/trn_rl_repo" not in sys.path:
    sys.path.insert(0, "/opt/trn_rl_repo")

import numpy as np
import ml_dtypes

import concourse.bass as bass  # noqa: F401
import concourse.tile as tile
from concourse import bacc, mybir
from concourse.bass_utils import run_bass_kernel_spmd
from concourse.masks import make_identity

BF16 = mybir.dt.bfloat16
F32 = mybir.dt.float32
AF = mybir.ActivationFunctionType

B, L, D = 4, 2048, 1024
P = 128
NQT = 8          # q-tiles per core, 128 rows each
SCALE = 1.0 / 32.0   # 1/sqrt(D)
NEG = -1.0e30
NWARM = 5        # PE warmup rounds spanning the head DMA wait

_CACHED = {}


def build_nc():
    nc = bacc.Bacc(None, target_bir_lowering=False)

    xt = nc.declare_dram_parameter("xt", [D, L], BF16, isOutput=False)    # x^T
    xr = nc.declare_dram_parameter("xr", [L, D], BF16, isOutput=False)    # x rows
    xtq = nc.declare_dram_parameter("xtq", [D, D], BF16, isOutput=False)  # q cols of x^T
    mm_w = nc.declare_dram_parameter("mm_w", [D, D], BF16, isOutput=False)  # Wq Wk^T
    wv = nc.declare_dram_parameter("wv", [D, D], BF16, isOutput=False)
    um = nc.declare_dram_parameter("um", [P, 8], F32, isOutput=False)     # Wk bq
    bvr = nc.declare_dram_parameter("bvr", [1, D], BF16, isOutput=False)
    mask = nc.declare_dram_parameter("mask", [P, 256], F32, isOutput=False)
    out = nc.declare_dram_parameter("out", [D, D], BF16, isOutput=True)

    with tile.TileContext(nc) as tc:
        with tc.tile_pool(name="persist", bufs=1) as persist:
            xt_sb = persist.tile([P, 8, L], BF16)    # x^T: [d-part, ct, token]
            xr_sb = persist.tile([P, 16, D], BF16)   # x: [tok-part, tt, d]
            gt_sb = persist.tile([P, 8, D], BF16)    # G'^T: [d-part, dt, qcol]
            xtq_sb = persist.tile([P, 8, D], BF16)
            m_sb = persist.tile([P, 8, D], BF16)
            wv_sb = persist.tile([P, 8, D], BF16)
            g1_sb = persist.tile([P, 8, 512], F32)   # phase-A pass-1 partials
            um_sb = persist.tile([P, 8], F32)
            bvr_sb = persist.tile([1, D], BF16)
            mask_sb = persist.tile([P, 256], F32)
            ident = persist.tile([P, P], BF16)
            ones_sb = persist.tile([1, P], BF16)
            ones_row = persist.tile([1, 512], BF16)  # DMA-free warmup operand
            warm_rhs = persist.tile([P, 512], BF16)  # full-K warmup operand
            bvb_sb = persist.tile([P, D], F32)       # broadcast bias 1 (x) bv

            make_identity(nc, ident)
            nc.vector.memset(ones_sb, 1.0)
            nc.vector.memset(ones_row, 1.0)
            nc.vector.memset(warm_rhs, 1.0)

            # DMA issue costs ~600ns of sequencer time PER dma_start, and an
            # engine pops its ready work strictly in emission order — so the
            # WHOLE priority-ordered input stream goes on the sync ring,
            # whose sequencer has nothing else to do until the output tiles.
            # The scalar engine issues NO DMAs (its copies pace phase A).
            nc.sync.dma_start(out=bvr_sb, in_=bvr[:, :])
            nc.sync.dma_start(out=um_sb, in_=um[:, :])
            for i in range(8):
                nc.sync.dma_start(out=m_sb[:, i, :], in_=mm_w[i * P:(i + 1) * P, :])
                nc.sync.dma_start(out=xtq_sb[:, i, :], in_=xtq[i * P:(i + 1) * P, :])
            nc.sync.dma_start(out=mask_sb, in_=mask[:, :])
            for c in range(2):
                for i in range(8):
                    nc.sync.dma_start(out=xt_sb[:, i, c * 1024:(c + 1) * 1024],
                                      in_=xt[i * P:(i + 1) * P, c * 1024:(c + 1) * 1024])
            # gpsimd (SWDGE) carries x rows and Wv, but must not steal queue
            # bandwidth from the critical stream above: each of its DMAs is
            # WAW-gated behind a 2-element vector write that reads the last
            # critical tile (xtq block 7), so descriptor generation only
            # starts once the sync stream has drained.
            for tt in range(16):
                nc.vector.tensor_copy(xr_sb[0:1, tt, 0:2], xtq_sb[0:1, 7, 0:2])
            for i in range(8):
                nc.vector.tensor_copy(wv_sb[0:1, i, 0:2], xtq_sb[0:1, 7, 0:2])
            for tt in range(4):
                nc.gpsimd.dma_start(out=xr_sb[:, tt, :], in_=xr[tt * P:(tt + 1) * P, :])
            for i in range(8):
                nc.gpsimd.dma_start(out=wv_sb[:, i, :], in_=wv[i * P:(i + 1) * P, :])
            for tt in range(4, 16):
                nc.gpsimd.dma_start(out=xr_sb[:, tt, :], in_=xr[tt * P:(tt + 1) * P, :])

            # Two PSUM pools spanning phases A and B (8 banks total):
            # psS: tags s (x2) + ptp (x2); psZ: pz0, pz1, po0, po1.
            with tc.tile_pool(name="bwork", bufs=2) as bwork, \
                 tc.tile_pool(name="psA", bufs=2, space="PSUM") as psS, \
                 tc.tile_pool(name="psC", bufs=1, space="PSUM") as psZ:
                psT = psS   # transposes share the psA pool (tag ptp)
                psO = psZ

                # Warmup matmuls keep the PE busy (and its clock ramping)
                # while the phase-A stream lands; early rounds drive the FULL
                # 128-deep array (ident lhsT) from memset constants so the
                # activity monitor sees real utilization and no DMA gates
                # them; the last round builds bvb = 1 (x) bv from the bv row.
                for r in range(NWARM):
                    for dc in range(2):
                        pb = psZ.tile([P, 512], F32, tag=f"po{dc}", name=f"pb{dc}")
                        if r == NWARM - 1:
                            nc.tensor.matmul(pb, ones_sb,
                                             bvr_sb[:, dc * 512:(dc + 1) * 512],
                                             start=True, stop=True)
                            nc.scalar.copy(bvb_sb[:, dc * 512:(dc + 1) * 512], pb)
                        else:
                            nc.tensor.matmul(pb, ident, warm_rhs,
                                             start=True, stop=True)

                # ------- Phase A: G'^T = M^T x_q^T + u (x) 1^T -------------
                # Contraction split 4+4 so pass 1 starts after half the
                # scalar-ring stream; pass 2 adds u via the affine bias slot.
                for qc in range(2):
                    qsl = slice(qc * 512, (qc + 1) * 512)
                    for dt in range(8):
                        pg = psS.tile([P, 512], F32, tag="s", name="pg")
                        for ct in range(4):
                            nc.tensor.matmul(
                                pg,
                                m_sb[:, ct, dt * P:(dt + 1) * P],
                                xtq_sb[:, ct, qsl],
                                start=(ct == 0),
                                stop=(ct == 3),
                            )
                        nc.vector.tensor_copy(g1_sb[:, dt, :], pg)
                    for dt in range(8):
                        pg = psS.tile([P, 512], F32, tag="s", name="pg2")
                        for ct in range(4, 8):
                            nc.tensor.matmul(
                                pg,
                                m_sb[:, ct, dt * P:(dt + 1) * P],
                                xtq_sb[:, ct, qsl],
                                start=(ct == 4),
                                stop=(ct == 7),
                            )
                        nc.vector.affine_then_add(
                            out=gt_sb[:, dt, qsl],
                            in0=pg,
                            in1=g1_sb[:, dt, :],
                            scale=1.0,
                            bias=um_sb[:, dt:dt + 1],
                        )

                # ------------- Phase B: attention per q-tile ---------------
                # S chunks -> exp (+rowsum) -> P^T -> Z = P x -> Z^T ->
                # O = Z^T.T Wv, scaled by 1/rowsum, + bvb.
                for t in range(NQT):
                    nk = 256 * (t + 1)
                    nkc = (nk + 511) // 512  # 512-chunks (last may be 256)

                    p_sb = bwork.tile([P, 2048], BF16, tag="p")
                    rsum = bwork.tile([P, 4], F32, tag="rsum")
                    pz = [psZ.tile([P, 512], F32, tag=f"pz{dc}", name=f"pz{dc}")
                          for dc in range(2)]

                    # For the last tile, process the diagonal chunk FIRST so
                    # its mask+exp chain hides under the other chunks' work
                    # instead of stalling the PE at the kernel tail.
                    corder = (list(range(nkc)) if t < NQT - 1
                              else [nkc - 1] + list(range(nkc - 1)))
                    first_kt = corder[0] * 4
                    lastc = corder[-1]
                    last_kt = lastc * 4 + min(512, nk - lastc * 512) // P - 1
                    for c in corder:
                        w = min(512, nk - c * 512)
                        ps = psS.tile([P, 512], F32, tag="s")
                        for dt in range(8):
                            nc.tensor.matmul(
                                ps[:, :w],
                                gt_sb[:, dt, t * P:(t + 1) * P],
                                xt_sb[:, dt, c * 512:c * 512 + w],
                                start=(dt == 0),
                                stop=(dt == 7),
                            )
                        if c == nkc - 1:
                            nc.vector.tensor_add(ps[:, w - 256:w],
                                                 ps[:, w - 256:w], mask_sb)
                        nc.scalar.activation(
                            p_sb[:, c * 512:c * 512 + w], ps[:, :w], AF.Exp,
                            scale=SCALE, accum_out=rsum[:, c:c + 1])
                        # All transposes+copies of this chunk first (copies
                        # alternate scalar/vector) so every Z matmul's
                        # LDWEIGHTS is prefetched instead of exposed.
                        kts = list(range(c * 4, c * 4 + w // P))
                        pts = {}
                        for j, kt in enumerate(kts):
                            ptp = psT.tile([P, P], BF16, tag="ptp")
                            nc.tensor.transpose(
                                ptp, p_sb[:, kt * P:(kt + 1) * P], ident)
                            pt_sb = bwork.tile([P, P], BF16, tag="pt", bufs=4)
                            if j % 2 == 0:
                                nc.vector.tensor_copy(pt_sb, ptp)
                            else:
                                nc.scalar.copy(pt_sb, ptp)
                            pts[kt] = pt_sb
                        for kt in kts:
                            for dc in range(2):
                                nc.tensor.matmul(
                                    pz[dc],
                                    pts[kt],
                                    xr_sb[:, kt, dc * 512:(dc + 1) * 512],
                                    start=(kt == first_kt),
                                    stop=(kt == last_kt),
                                )

                    # Z -> SBUF (bf16) in 256-col chunks split across the
                    # scalar+vector engines (keeps exp off the copy queue),
                    # then Z^T tiles, then O = Z Wv
                    z_sb = bwork.tile([P, D], BF16, tag="z")
                    for zc in range(4):
                        src = pz[zc // 2][:, (zc % 2) * 256:(zc % 2) * 256 + 256]
                        dst = z_sb[:, zc * 256:(zc + 1) * 256]
                        if zc % 2 == 0:
                            nc.vector.tensor_copy(dst, src)
                        else:
                            nc.scalar.copy(dst, src)
                    po = [psO.tile([P, 512], F32, tag=f"po{dc}", name=f"po{dc}")
                          for dc in range(2)]
                    zts = []
                    for cc in range(8):
                        ztp = psT.tile([P, P], BF16, tag="ptp", name="ztp")
                        nc.tensor.transpose(ztp, z_sb[:, cc * P:(cc + 1) * P], ident)
                        zt_sb = bwork.tile([P, P], BF16, tag="zt", bufs=8)
                        if cc % 2 == 0:
                            nc.vector.tensor_copy(zt_sb, ztp)
                        else:
                            nc.scalar.copy(zt_sb, ztp)
                        zts.append(zt_sb)

                    rinv = bwork.tile([P, 1], F32, tag="rinv")
                    rtot = bwork.tile([P, 1], F32, tag="rtot")
                    nc.vector.reduce_sum(rtot, rsum[:, :nkc], axis=mybir.AxisListType.X)
                    nc.vector.reciprocal(rinv, rtot)
                    o_sb = bwork.tile([P, D], BF16, tag="o")
                    if t < NQT - 1:
                        for cc in range(8):
                            for dc in range(2):
                                nc.tensor.matmul(
                                    po[dc], zts[cc],
                                    wv_sb[:, cc, dc * 512:(dc + 1) * 512],
                                    start=(cc == 0), stop=(cc == 7))
                        for dc in range(2):
                            sl = slice(dc * 512, (dc + 1) * 512)
                            nc.vector.affine_then_add(
                                out=o_sb[:, sl], in0=po[dc], in1=bvb_sb[:, sl],
                                scale=rinv, bias=0.0)
                            nc.sync.dma_start(out=out[t * P:(t + 1) * P, sl],
                                              in_=o_sb[:, sl])
                    else:
                        # Last tile: finish po[0] first so its normalize +
                        # store overlap po[1]'s matmuls (shorter tail).
                        for dc in range(2):
                            for cc in range(8):
                                nc.tensor.matmul(
                                    po[dc], zts[cc],
                                    wv_sb[:, cc, dc * 512:(dc + 1) * 512],
                                    start=(cc == 0), stop=(cc == 7))
                            sl = slice(dc * 512, (dc + 1) * 512)
                            nc.vector.affine_then_add(
                                out=o_sb[:, sl], in0=po[dc], in1=bvb_sb[:, sl],
                                scale=rinv, bias=0.0)
                            nc.sync.dma_start(out=out[t * P:(t + 1) * P, sl],
                                              in_=o_sb[:, sl])

    nc.finalize()
    return nc


def _prep_inputs(x, wq, bq, wk, bk, wv, bv):
    bf = ml_dtypes.bfloat16
    wq32 = np.asarray(wq, np.float32)
    wk32 = np.asarray(wk, np.float32)
    m_host = (wq32 @ wk32.T).astype(bf)                 # Wq Wk^T
    u_host = (wk32 @ np.asarray(bq, np.float32))        # Wk bq, [D]
    um = np.ascontiguousarray(u_host.reshape(8, P).T).astype(np.float32)
    wv_b = np.ascontiguousarray(wv, np.float32).astype(bf)
    bvr = np.asarray(bv, np.float32).reshape(1, D).astype(bf)

    i = np.arange(P)[:, None]
    j = np.arange(256)[None, :]
    masks = [np.where(j <= i + P * h, 0.0, NEG).astype(np.float32)
             for h in range(2)]

    in_maps = []
    for core in range(8):
        b, h = core // 2, core % 2
        xb = np.asarray(x[b], np.float32)
        xT = np.ascontiguousarray(xb.T).astype(bf)
        xR = xb.astype(bf)
        qcols = (np.arange(8)[:, None] * 2 + h) * P + np.arange(P)[None, :]
        xTq = np.ascontiguousarray(xT[:, qcols.ravel()])
        in_maps.append({
            "xt": xT, "xr": xR, "xtq": xTq, "mm_w": m_host, "wv": wv_b,
            "um": um, "bvr": bvr, "mask": masks[h],
        })
    return in_maps


def kernel(x, wq, bq, wk, bk, wv, bv, _trace=False, _trace_kwargs=None):
    if "nc" not in _CACHED:
        _CACHED["nc"] = build_nc()
    nc = _CACHED["nc"]
    in_maps = _prep_inputs(x, wq, bq, wk, bk, wv, bv)
    kw = {}
    if _trace:
        kw = dict(trace=True, **(_trace_kwargs or {}))
    res = run_bass_kernel_spmd(nc, in_maps, list(range(8)), **kw)
    out = np.empty((B, L, D), np.float32)
    for core in range(8):
        b, h = core // 2, core % 2
        o = np.asarray(res.results[core]["out"], np.float32)
        out[b].reshape(16, P, D)[h::2] = o.reshape(NQT, P, D)
    if _trace:
        _CACHED["last_results"] = res
    return out
